# revision 8
# baseline (speedup 1.0000x reference)
"""Batch neighbor-list kernel for Trainium2 (8 NeuronCores, SPMD).

Problem: for B=4 systems of N=512 atoms each, compute for every pair (i,j)
and each of 27 periodic image shifts s:
    diff[i,j,s,:] = (wrap(pos)[j] + shift_s @ cell) - wrap(pos)[i]
    dist[i,j,s]   = |diff|
    mask          = (dist < 5.0) & (dist > 0.01)
returning (diff*mask, dist*mask, mask) with shapes
[4,512,512,27,3] f32, [4,512,512,27] f32, [4,512,512,27] bool.

Sharding: 8 cores = (4 systems) x (2 halves of the i axis). Each core
produces a [256, 512, 27] block independently; no cross-core communication.

Device pipeline per core (i on partitions, (j,s,k) on the free axis):
  - PE matmul with contraction dim 4 computes diff = A[j,s,k] - wp[i,k]
    directly into PSUM (rhs rows: A and the k-one-hot pattern; stationary
    lhsT rows: [1, -wp_x, -wp_y, -wp_z]).
  - ScalarE: squares of the three k-strided diff components, and sqrt(d2).
  - VectorE: d2 = sq_x + sq_y, mask = (d2 < 25) as f32, and the fused
    TENSOR_MASK custom DVE op out_diff = select(d2 < 25, diff, 0).
  - GPSIMD: d2 += sq_z and out_dist = dist * mask.
  - DMA out via HWDGE; the bool mask is cast f32->u8 in a SWDGE DMA.

The (dist > 0.01) lower bound only excludes the exact self-pair
(i == j, s == center): it is fixed up on the host (mask[b,i,i,13] = False).
"""

import numpy as np

import bass_rust
import concourse.bass as bass
import concourse.mybir as mybir
from concourse.bass_utils import run_bass_kernel_spmd
from concourse.tile import TileContext

B, N, S = 4, 512, 27
NCORES = 8
IH = N // 2          # 256 i-rows per core
P = 128              # SBUF partitions
ITILES = IH // P     # 2 i-tiles per core
CUT2 = 25.0          # CUTOFF**2
EPS = 1e-7
CENTER_SHIFT = 13    # shifts[13] == (0,0,0)

FREE_DIST = N * S        # 13824 (j,s) pairs per i-row
FREE_DIFF = N * S * 3    # 41472
SC_DIST = 512            # (j,s) pairs per super-chunk (one PSUM bank per matmul)
SC_DIFF = SC_DIST * 3    # 1536 diff elements (3 PSUM banks)
NSC = FREE_DIFF // SC_DIFF   # 27 super-chunks per i-tile
G_SC = 3                 # super-chunks per DMA group
NG = NSC // G_SC         # 9 groups per i-tile
GD_DIST = G_SC * SC_DIST     # 1536
GD_DIFF = G_SC * SC_DIFF     # 4608

F32 = mybir.dt.float32

LAST_RESULT = None
_BASS_CACHE = {}

# Instruction formats embed a limited number of sync waits (fp32 Matmult and
# NoOp hold exactly one; DMACopy fewer than three). Walrus codegen hard-fails
# on excess instead of splitting, so after Tile scheduling we hoist all but
# the last wait of each instruction onto single-wait NoOps on the same engine
# queue, which preserves semantics (the queue stalls at the NoOp instead).
_NO_SPLIT = (
    mybir.InstEventSemaphore,
    mybir.InstAllEngineBarrier,
)


def _split_waits(nc):
    for fn in nc.m.functions:
        for blk in fn.blocks:
            out = []
            for ins in blk.instructions:
                si = ins.sync_info
                if (
                    si is not None
                    and not isinstance(ins, _NO_SPLIT)
                    and ins.engine is not None
                    and len(si.on_wait) > 1
                ):
                    waits = list(si.on_wait)
                    for j, w in enumerate(waits[:-1]):
                        out.append(
                            mybir.InstNoOp(
                                name=f"{ins.name}-wsplit{j}",
                                engine=ins.engine,
                                text_hint="wsplit",
                                bass_nofuse=True,
                                sync_info=bass_rust.SyncInfo(
                                    on_wait=[w], on_update=[]
                                ),
                            )
                        )
                    si.on_wait = [waits[-1]]
                out.append(ins)
            blk.instructions[:] = out


def _build_bass():
    nc = bass.Bass()
    X = nc.dram_tensor("X", [4, FREE_DIFF], F32, kind="ExternalInput")
    W = nc.dram_tensor("W", [4, IH], F32, kind="ExternalInput")
    out_diff = nc.dram_tensor("out_diff", [IH, FREE_DIFF], F32, kind="ExternalOutput")
    out_dist = nc.dram_tensor("out_dist", [IH, FREE_DIST], F32, kind="ExternalOutput")
    out_mask = nc.dram_tensor(
        "out_mask", [IH, FREE_DIST], mybir.dt.uint8, kind="ExternalOutput"
    )

    with TileContext(nc) as tc:
        with (
            tc.tile_pool(name="wpool", bufs=2) as wpool,
            tc.tile_pool(name="xpool", bufs=3) as xpool,
            tc.tile_pool(name="sqpool", bufs=3) as sqpool,
            tc.tile_pool(name="spool", bufs=4) as spool,
            tc.tile_pool(name="diffpool", bufs=2) as diffpool,
            tc.tile_pool(name="distpool", bufs=2) as distpool,
            tc.tile_pool(name="maskpool", bufs=2) as maskpool,
            tc.tile_pool(name="psum", bufs=2, space="PSUM") as psumpool,
        ):
            for it in range(ITILES):
                w_sb = wpool.tile([4, P], F32, tag="w")
                nc.sync.dma_start(out=w_sb[:], in_=W[:, it * P : (it + 1) * P])
                for g in range(NG):
                    diff_buf = diffpool.tile([P, GD_DIFF], F32, tag="diffbuf")
                    dist_buf = distpool.tile([P, GD_DIST], F32, tag="distbuf")
                    m_buf = maskpool.tile([P, GD_DIST], F32, tag="mbuf")
                    diff3 = diff_buf[:].rearrange("p (n k) -> p n k", k=3)
                    for u in range(G_SC):
                        scg = g * G_SC + u
                        x_sb = xpool.tile([4, SC_DIFF], F32, tag="xsb")
                        nc.sync.dma_start(
                            out=x_sb[:],
                            in_=X[:, scg * SC_DIFF : (scg + 1) * SC_DIFF],
                        )
                        ps = psumpool.tile([P, SC_DIFF], F32, tag="ps")
                        for c in range(3):
                            nc.tensor.matmul(
                                ps[:, c * 512 : (c + 1) * 512],
                                w_sb[:],
                                x_sb[:, c * 512 : (c + 1) * 512],
                            )
                        ps3 = ps[:].rearrange("p (n k) -> p n k", k=3)
                        sq = sqpool.tile([P, SC_DIFF], F32, tag="sq")
                        nc.scalar.activation(
                            sq[:], ps[:], mybir.ActivationFunctionType.Square
                        )
                        sq3 = sq[:].rearrange("p (n k) -> p n k", k=3)
                        d2a = spool.tile([P, SC_DIST], F32, tag="d2a")
                        nc.vector.tensor_add(d2a[:], sq3[:, :, 0], sq3[:, :, 1])
                        d2 = spool.tile([P, SC_DIST], F32, tag="d2")
                        nc.gpsimd.tensor_tensor(
                            d2[:], d2a[:], sq3[:, :, 2], mybir.AluOpType.add
                        )
                        moff = u * SC_DIST
                        m_sl = m_buf[:, moff : moff + SC_DIST]
                        nc.vector.tensor_scalar(
                            m_sl, d2[:], CUT2, None, mybir.AluOpType.is_lt
                        )
                        dst = spool.tile([P, SC_DIST], F32, tag="dst")
                        nc.scalar.activation(
                            dst[:], d2[:], mybir.ActivationFunctionType.Sqrt
                        )
                        nc.gpsimd.tensor_tensor(
                            dist_buf[:, moff : moff + SC_DIST],
                            dst[:],
                            m_sl,
                            mybir.AluOpType.mult,
                        )
                        # out_diff = diff * mask (mask broadcast over k)
                        m_b = m_sl[:, :, None].broadcast_to([P, SC_DIST, 3])
                        d3_sl = diff3[:, u * SC_DIST : (u + 1) * SC_DIST, :]
                        nc.vector.tensor_tensor(
                            d3_sl, ps3[:], m_b, mybir.AluOpType.mult
                        )
                    ro = it * P
                    nc.sync.dma_start(
                        out=out_diff[ro : ro + P, g * GD_DIFF : (g + 1) * GD_DIFF],
                        in_=diff_buf[:],
                    )
                    nc.sync.dma_start(
                        out=out_dist[ro : ro + P, g * GD_DIST : (g + 1) * GD_DIST],
                        in_=dist_buf[:],
                    )
                    # SWDGE DMA casts f32 -> u8 in flight
                    nc.gpsimd.dma_start(
                        out=out_mask[ro : ro + P, g * GD_DIST : (g + 1) * GD_DIST],
                        in_=m_buf[:],
                    )
    _split_waits(nc)
    return nc


def _get_bass():
    if "nc" not in _BASS_CACHE:
        _BASS_CACHE["nc"] = _build_bass()
    return _BASS_CACHE["nc"]


def _host_prep(positions, cell):
    """Wrapped positions and A[j,s,k] = wp[j]+shift_s@cell, mirroring the
    reference op order in float32."""
    pos = positions.reshape(B, N, 3)
    r = np.arange(-1, 2)
    shifts = (
        np.stack(np.meshgrid(r, r, r, indexing="ij"), axis=-1)
        .reshape(-1, 3)
        .astype(np.float32)
    )
    wp = np.empty((B, N, 3), np.float32)
    A = np.empty((B, N, S, 3), np.float32)
    half = np.float32(0.5)
    eps = np.float32(EPS)
    for b in range(B):
        inv = np.linalg.inv(cell[b]).astype(np.float32)
        t = pos[b] @ inv + eps
        # jax's f32 `% 1.0` in this environment lowers to the round-nearest
        # remainder x - floor(x + 0.5); mirror it exactly, NOT np.mod.
        scaled = (t - np.floor(t + half)) - eps
        wp[b] = (scaled @ cell[b]).astype(np.float32)
        sc = (shifts @ cell[b]).astype(np.float32)
        A[b] = wp[b][:, None, :] + sc[None, :, :]
    return wp, A


def kernel(positions, cell, n_atoms):
    global LAST_RESULT
    positions = np.ascontiguousarray(np.asarray(positions, dtype=np.float32))
    cell = np.ascontiguousarray(np.asarray(cell, dtype=np.float32))

    wp, A = _host_prep(positions, cell)

    onehot = np.zeros((3, FREE_DIFF), np.float32)
    for kk in range(3):
        onehot[kk, kk::3] = 1.0

    in_maps = []
    for c in range(NCORES):
        b, h = divmod(c, 2)
        X = np.empty((4, FREE_DIFF), np.float32)
        X[0] = A[b].reshape(-1)
        X[1:] = onehot
        Wm = np.empty((4, IH), np.float32)
        Wm[0] = 1.0
        Wm[1:] = -wp[b, h * IH : (h + 1) * IH].T
        in_maps.append({"X": X, "W": Wm})

    nc = _get_bass()
    res = run_bass_kernel_spmd(nc, in_maps, core_ids=list(range(NCORES)))
    LAST_RESULT = res

    diff = np.empty((B, N, N, S, 3), np.float32)
    dist = np.empty((B, N, N, S), np.float32)
    mask = np.empty((B, N, N, S), np.uint8)
    for c in range(NCORES):
        b, h = divmod(c, 2)
        o = res.results[c]
        diff[b, h * IH : (h + 1) * IH] = o["out_diff"].reshape(IH, N, S, 3)
        dist[b, h * IH : (h + 1) * IH] = o["out_dist"].reshape(IH, N, S)
        mask[b, h * IH : (h + 1) * IH] = o["out_mask"].reshape(IH, N, S)

    # (dist > MIN_DIST) only excludes the exact self-pair; diff/dist are
    # already exactly 0 there, only the mask needs the fixup.
    ar = np.arange(N)
    mask[:, ar, ar, CENTER_SHIFT] = 0
    diff[:, ar, ar, CENTER_SHIFT] = 0.0
    dist[:, ar, ar, CENTER_SHIFT] = 0.0
    return diff, dist, mask.astype(bool)


# revision 11
# speedup vs baseline: 1.3975x; 1.3975x over previous
"""Batch neighbor-list kernel for Trainium2 (8 NeuronCores, SPMD).

Problem: for B=4 systems of N=512 atoms each, compute for every pair (i,j)
and each of 27 periodic image shifts s:
    diff[i,j,s,:] = (wrap(pos)[j] + shift_s @ cell) - wrap(pos)[i]
    dist[i,j,s]   = |diff|
    mask          = (dist < 5.0) & (dist > 0.01)
returning (diff*mask, dist*mask, mask) with shapes
[4,512,512,27,3] f32, [4,512,512,27] f32, [4,512,512,27] bool.

Sharding: 8 cores = (4 systems) x (2 halves of the i axis). Each core
produces a [256, 512, 27] block independently; no cross-core communication.

Device pipeline per core (i on partitions, (j,s,k) on the free axis):
  - PE matmul with contraction dim 4 computes diff = A[j,s,k] - wp[i,k]
    directly into PSUM (rhs rows: A and the k-one-hot pattern; stationary
    lhsT rows: [1, -wp_x, -wp_y, -wp_z]).
  - ScalarE: squares of the three k-strided diff components, and sqrt(d2).
  - VectorE: d2 = sq_x + sq_y, mask = (d2 < 25) as f32, and the fused
    TENSOR_MASK custom DVE op out_diff = select(d2 < 25, diff, 0).
  - GPSIMD: d2 += sq_z and out_dist = dist * mask.
  - DMA out via HWDGE; the bool mask is cast f32->u8 in a SWDGE DMA.

The (dist > 0.01) lower bound only excludes the exact self-pair
(i == j, s == center): it is fixed up on the host (mask[b,i,i,13] = False).
"""

import numpy as np

import bass_rust
import concourse.bass as bass
import concourse.mybir as mybir
from concourse.bass_utils import run_bass_kernel_spmd
from concourse.tile import TileContext

B, N, S = 4, 512, 27
NCORES = 8
IH = N // 2          # 256 i-rows per core
P = 128              # SBUF partitions
ITILES = IH // P     # 2 i-tiles per core
CUT2 = 25.0          # CUTOFF**2
EPS = 1e-7
CENTER_SHIFT = 13    # shifts[13] == (0,0,0)

FREE_DIST = N * S        # 13824 (j,s) pairs per i-row
FREE_DIFF = N * S * 3    # 41472
SC_DIST = 512            # (j,s) pairs per super-chunk (one PSUM bank per matmul)
SC_DIFF = SC_DIST * 3    # 1536 diff elements (3 PSUM banks)
NSC = FREE_DIFF // SC_DIFF   # 27 super-chunks per i-tile
G_SC = 3                 # super-chunks per DMA group
NG = NSC // G_SC         # 9 groups per i-tile
GD_DIST = G_SC * SC_DIST     # 1536
GD_DIFF = G_SC * SC_DIFF     # 4608

KDIM = 12            # 3-way bf16 split of A plus 3x3 split of -wp
F32 = mybir.dt.float32

LAST_RESULT = None
_BASS_CACHE = {}

# Instruction formats embed a limited number of sync waits (fp32 Matmult and
# NoOp hold exactly one; DMACopy fewer than three). Walrus codegen hard-fails
# on excess instead of splitting, so after Tile scheduling we hoist all but
# the last wait of each instruction onto single-wait NoOps on the same engine
# queue, which preserves semantics (the queue stalls at the NoOp instead).
_NO_SPLIT = (
    mybir.InstEventSemaphore,
    mybir.InstAllEngineBarrier,
)


def _split_waits(nc):
    for fn in nc.m.functions:
        for blk in fn.blocks:
            out = []
            for ins in blk.instructions:
                si = ins.sync_info
                if (
                    si is not None
                    and not isinstance(ins, _NO_SPLIT)
                    and ins.engine is not None
                    and len(si.on_wait) > 1
                ):
                    waits = list(si.on_wait)
                    for j, w in enumerate(waits[:-1]):
                        out.append(
                            mybir.InstNoOp(
                                name=f"{ins.name}-wsplit{j}",
                                engine=ins.engine,
                                text_hint="wsplit",
                                bass_nofuse=True,
                                sync_info=bass_rust.SyncInfo(
                                    on_wait=[w], on_update=[]
                                ),
                            )
                        )
                    si.on_wait = [waits[-1]]
                out.append(ins)
            blk.instructions[:] = out


def _build_bass():
    nc = bass.Bass()
    BF16 = mybir.dt.bfloat16
    X = nc.dram_tensor("X", [KDIM, FREE_DIFF], BF16, kind="ExternalInput")
    W = nc.dram_tensor("W", [KDIM, IH], BF16, kind="ExternalInput")
    out_diff = nc.dram_tensor("out_diff", [IH, FREE_DIFF], F32, kind="ExternalOutput")
    out_dist = nc.dram_tensor("out_dist", [IH, FREE_DIST], F32, kind="ExternalOutput")
    out_mask = nc.dram_tensor(
        "out_mask", [IH, FREE_DIST], mybir.dt.uint8, kind="ExternalOutput"
    )

    with TileContext(nc) as tc:
        with (
            tc.tile_pool(name="wpool", bufs=2) as wpool,
            tc.tile_pool(name="xpool", bufs=3) as xpool,
            tc.tile_pool(name="sqpool", bufs=3) as sqpool,
            tc.tile_pool(name="spool", bufs=4) as spool,
            tc.tile_pool(name="diffpool", bufs=2) as diffpool,
            tc.tile_pool(name="distpool", bufs=2) as distpool,
            tc.tile_pool(name="maskpool", bufs=2) as maskpool,
            tc.tile_pool(name="psum", bufs=2, space="PSUM") as psumpool,
        ):
            for it in range(ITILES):
                w_sb = wpool.tile([KDIM, P], BF16, tag="w")
                nc.sync.dma_start(out=w_sb[:], in_=W[:, it * P : (it + 1) * P])
                for g in range(NG):
                    diff_buf = diffpool.tile([P, GD_DIFF], F32, tag="diffbuf")
                    dist_buf = distpool.tile([P, GD_DIST], F32, tag="distbuf")
                    m_buf = maskpool.tile([P, GD_DIST], F32, tag="mbuf")
                    diff3 = diff_buf[:].rearrange("p (n k) -> p n k", k=3)
                    for u in range(G_SC):
                        scg = g * G_SC + u
                        x_sb = xpool.tile([KDIM, SC_DIFF], BF16, tag="xsb")
                        nc.sync.dma_start(
                            out=x_sb[:],
                            in_=X[:, scg * SC_DIFF : (scg + 1) * SC_DIFF],
                        )
                        ps = psumpool.tile([P, SC_DIFF], F32, tag="ps")
                        for c in range(3):
                            nc.tensor.matmul(
                                ps[:, c * 512 : (c + 1) * 512],
                                w_sb[:],
                                x_sb[:, c * 512 : (c + 1) * 512],
                            )
                        ps3 = ps[:].rearrange("p (n k) -> p n k", k=3)
                        sq = sqpool.tile([P, SC_DIFF], F32, tag="sq")
                        nc.scalar.activation(
                            sq[:], ps[:], mybir.ActivationFunctionType.Square
                        )
                        sq3 = sq[:].rearrange("p (n k) -> p n k", k=3)
                        d2a = spool.tile([P, SC_DIST], F32, tag="d2a")
                        nc.vector.tensor_add(d2a[:], sq3[:, :, 0], sq3[:, :, 1])
                        d2 = spool.tile([P, SC_DIST], F32, tag="d2")
                        nc.gpsimd.tensor_tensor(
                            d2[:], d2a[:], sq3[:, :, 2], mybir.AluOpType.add
                        )
                        moff = u * SC_DIST
                        m_sl = m_buf[:, moff : moff + SC_DIST]
                        nc.vector.tensor_scalar(
                            m_sl, d2[:], CUT2, None, mybir.AluOpType.is_lt
                        )
                        dst = spool.tile([P, SC_DIST], F32, tag="dst")
                        nc.scalar.activation(
                            dst[:], d2[:], mybir.ActivationFunctionType.Sqrt
                        )
                        nc.gpsimd.tensor_tensor(
                            dist_buf[:, moff : moff + SC_DIST],
                            dst[:],
                            m_sl,
                            mybir.AluOpType.mult,
                        )
                        # out_diff = diff * mask (mask broadcast over k)
                        m_b = m_sl[:, :, None].broadcast_to([P, SC_DIST, 3])
                        d3_sl = diff3[:, u * SC_DIST : (u + 1) * SC_DIST, :]
                        nc.vector.tensor_tensor(
                            d3_sl, ps3[:], m_b, mybir.AluOpType.mult
                        )
                    ro = it * P
                    nc.sync.dma_start(
                        out=out_diff[ro : ro + P, g * GD_DIFF : (g + 1) * GD_DIFF],
                        in_=diff_buf[:],
                    )
                    nc.sync.dma_start(
                        out=out_dist[ro : ro + P, g * GD_DIST : (g + 1) * GD_DIST],
                        in_=dist_buf[:],
                    )
                    # SWDGE DMA casts f32 -> u8 in flight
                    nc.gpsimd.dma_start(
                        out=out_mask[ro : ro + P, g * GD_DIST : (g + 1) * GD_DIST],
                        in_=m_buf[:],
                    )
    _split_waits(nc)
    return nc


def _get_bass():
    if "nc" not in _BASS_CACHE:
        _BASS_CACHE["nc"] = _build_bass()
    return _BASS_CACHE["nc"]


def _host_prep(positions, cell):
    """Wrapped positions and A[j,s,k] = wp[j]+shift_s@cell, mirroring the
    reference op order in float32."""
    pos = positions.reshape(B, N, 3)
    r = np.arange(-1, 2)
    shifts = (
        np.stack(np.meshgrid(r, r, r, indexing="ij"), axis=-1)
        .reshape(-1, 3)
        .astype(np.float32)
    )
    wp = np.empty((B, N, 3), np.float32)
    A = np.empty((B, N, S, 3), np.float32)
    half = np.float32(0.5)
    eps = np.float32(EPS)
    for b in range(B):
        inv = np.linalg.inv(cell[b]).astype(np.float32)
        t = pos[b] @ inv + eps
        # jax's f32 `% 1.0` in this environment lowers to the round-nearest
        # remainder x - floor(x + 0.5); mirror it exactly, NOT np.mod.
        scaled = (t - np.floor(t + half)) - eps
        wp[b] = (scaled @ cell[b]).astype(np.float32)
        sc = (shifts @ cell[b]).astype(np.float32)
        A[b] = wp[b][:, None, :] + sc[None, :, :]
    return wp, A


def kernel(positions, cell, n_atoms):
    global LAST_RESULT
    positions = np.ascontiguousarray(np.asarray(positions, dtype=np.float32))
    cell = np.ascontiguousarray(np.asarray(cell, dtype=np.float32))

    wp, A = _host_prep(positions, cell)

    import ml_dtypes

    bf16 = ml_dtypes.bfloat16

    def split3(v):
        """Exact 3-way bf16 split: v == hi + mid + lo bitwise in f32."""
        hi = v.astype(bf16)
        r1 = (v - hi.astype(np.float32)).astype(np.float32)
        mid = r1.astype(bf16)
        lo = (r1 - mid.astype(np.float32)).astype(np.float32).astype(bf16)
        return hi, mid, lo

    onehot = np.zeros((3, FREE_DIFF), bf16)
    for kk in range(3):
        onehot[kk, kk::3] = 1.0

    in_maps = []
    for c in range(NCORES):
        b, h = divmod(c, 2)
        X = np.empty((KDIM, FREE_DIFF), bf16)
        X[0], X[1], X[2] = split3(A[b].reshape(-1))
        for kk in range(3):
            X[3 + 3 * kk : 6 + 3 * kk] = onehot[kk]
        Wm = np.empty((KDIM, IH), bf16)
        Wm[0:3] = 1.0
        nwp = -wp[b, h * IH : (h + 1) * IH]  # [IH, 3]
        for kk in range(3):
            hi, mid, lo = split3(nwp[:, kk])
            Wm[3 + 3 * kk] = hi
            Wm[4 + 3 * kk] = mid
            Wm[5 + 3 * kk] = lo
        in_maps.append({"X": X, "W": Wm})

    nc = _get_bass()
    res = run_bass_kernel_spmd(nc, in_maps, core_ids=list(range(NCORES)))
    LAST_RESULT = res

    diff = np.empty((B, N, N, S, 3), np.float32)
    dist = np.empty((B, N, N, S), np.float32)
    mask = np.empty((B, N, N, S), np.uint8)
    for c in range(NCORES):
        b, h = divmod(c, 2)
        o = res.results[c]
        diff[b, h * IH : (h + 1) * IH] = o["out_diff"].reshape(IH, N, S, 3)
        dist[b, h * IH : (h + 1) * IH] = o["out_dist"].reshape(IH, N, S)
        mask[b, h * IH : (h + 1) * IH] = o["out_mask"].reshape(IH, N, S)

    # (dist > MIN_DIST) only excludes the exact self-pair; diff/dist are
    # already exactly 0 there, only the mask needs the fixup.
    ar = np.arange(N)
    mask[:, ar, ar, CENTER_SHIFT] = 0
    diff[:, ar, ar, CENTER_SHIFT] = 0.0
    dist[:, ar, ar, CENTER_SHIFT] = 0.0
    return diff, dist, mask.astype(bool)
